# revision 13
# baseline (speedup 1.0000x reference)
"""Bass/Tile kernel for nn_CustomCrossAttnProcessor (8-core data-parallel).

Each NeuronCore processes one batch element (B=8 == n_cores).
Per-core compute, one batch element:
  q = hs @ w_q                       (f32r matmuls, N=256)
  k/v = enc @ w_{k,v}; ip_k/ip_v = ip @ w_{k,v}_ip   (setup)
  merged scores [112, 512] per head-pair: txt rows 0:77, ip rows 96:112,
  one exp per pair (no max-subtract: |scores*scale| <= ~5), probs bf16
  PV with ones/vrowsum columns appended to v -> softmax denominators and
  row-means for free; all 20 heads of one (si, branch) live in two
  2-bank psum tiles -> one big normalize op per group
  norm_ipa stats via tensor_tensor_reduce; -g*wsum + bias folded into the
  out-projection as an extra f32r matmul; out = hs_sum @ w_out (bf16).
"""
import sys

for _p in ("/opt/trn_rl_repo",):
    if _p not in sys.path:
        sys.path.append(_p)

from contextlib import ExitStack

import numpy as np

import concourse.bass as bass  # noqa: F401
import concourse.tile as tile
import concourse.mybir as mybir
from concourse import bass_utils, bacc
from concourse.bass import ts, ds
from concourse.masks import make_identity

B, S, D = 8, 4096, 1280
T, P_IP, C = 77, 16, 2048
H, HD = 20, 64
SB = 256            # tokens per s-block
NBLK = S // SB      # 16
SCALE = HD ** -0.5  # 0.125
EPS = 1e-7
KD = D // 128       # 10
KC = C // 128       # 16
CAT = 112           # score rows: txt [0:77], zero pad [77:96], ip [96:112]
IPOFF = 96
VW = HD + 2         # 66: v cols + ones col (softmax sum) + v-rowsum col
ALU = mybir.AluOpType
FT = mybir.ActivationFunctionType
AXX = mybir.AxisListType.X

f32 = mybir.dt.float32
f32r = mybir.dt.float32r
bf16 = mybir.dt.bfloat16

_CACHE = {}


def _build():
    nc = bacc.Bacc(
        "TRN2", target_bir_lowering=False, debug=False, enable_asserts=False,
        num_devices=8,
    )
    hs_d = nc.dram_tensor("hidden_states", [S, D], f32, kind="ExternalInput").ap()
    enc_d = nc.dram_tensor("encoder_hidden_states", [T, C], f32,
                           kind="ExternalInput").ap()
    ip_d = nc.dram_tensor("ip_hidden_states", [P_IP, C], f32,
                          kind="ExternalInput").ap()
    wq_d = nc.dram_tensor("w_q", [D, D], f32r, kind="ExternalInput").ap()
    wk_d = nc.dram_tensor("w_k", [C, D], f32r, kind="ExternalInput").ap()
    wv_d = nc.dram_tensor("w_v", [C, D], f32r, kind="ExternalInput").ap()
    wkip_d = nc.dram_tensor("w_k_ip", [C, D], f32r, kind="ExternalInput").ap()
    wvip_d = nc.dram_tensor("w_v_ip", [C, D], f32r, kind="ExternalInput").ap()
    wout_d = nc.dram_tensor("w_out", [D, D], f32, kind="ExternalInput").ap()
    bout_d = nc.dram_tensor("b_out", [D], f32, kind="ExternalInput").ap()
    out_d = nc.dram_tensor("out", [S, D], f32, kind="ExternalOutput").ap()

    with tile.TileContext(nc) as tc, ExitStack() as ctx:
        n = tc.nc
        const = ctx.enter_context(tc.tile_pool(name="const", bufs=1))
        wq_sb = const.tile([128, KD, D], f32r)
        wout_sb = const.tile([128, KD, D], bf16)
        ktc_sb = const.tile([128, KD, CAT], bf16)
        vaug_sb = const.tile([128, H, VW], bf16)   # rows 0:77 hold txt v
        ipv_sb = const.tile([128, H, VW], bf16)    # rows 96:112 hold ip v
        ident = const.tile([128, 128], f32)
        identb = const.tile([128, 128], bf16)
        onesb_col = const.tile([128, 1], bf16)
        b_row = const.tile([1, D], f32)
        gw128 = const.tile([128, D], f32r)  # row0=-colsum(w_out), row1=b_out

        make_identity(n, ident[:])
        make_identity(n, identb[:])
        n.vector.memset(onesb_col[:], 1.0)
        n.vector.memset(ktc_sb[:, :, T:IPOFF], 0.0)
        n.vector.memset(vaug_sb[0:T, :, HD:HD + 1], 1.0)
        n.vector.memset(ipv_sb[IPOFF:IPOFF + P_IP, :, HD:HD + 1], 1.0)
        n.vector.memset(gw128.bitcast(f32)[:], 0.0)
        n.sync.dma_start(wq_sb[:], wq_d.rearrange("(ko ki) m -> ki ko m", ki=128))
        n.sync.dma_start(b_row[:], bout_d[None, :])
        n.vector.tensor_copy(gw128.bitcast(f32)[1:2, :], b_row[:])

        # ---------------- setup: weights, k/v projections -------------------
        with tc.tile_pool(name="setup", bufs=1) as setup, \
             tc.tile_pool(name="setup_w", bufs=3) as setup_w:
          with tc.tile_pool(name="sps1", bufs=2, space="PSUM") as sps1:
            # wout: load f32 (two halves), cast to bf16
            for half in range(2):
                wout_f = setup.tile([128, KD // 2, D], f32, tag="woutf")
                n.sync.dma_start(
                    wout_f[:],
                    wout_d.rearrange("(ko ki) m -> ki ko m", ki=128)[
                        :, ds(half * (KD // 2), KD // 2), :])
                for k in range(KD // 2):
                    n.scalar.copy(wout_sb[:, half * (KD // 2) + k, :],
                                  wout_f[:, k, :])
            # column sums of w_out for the -g*wsum fold
            for j in range(3):
                w = min(512, D - j * 512)
                wsp = sps1.tile([1, 512], f32, tag="wsp", bufs=1)
                for k in range(KD):
                    n.tensor.matmul(wsp[:, :w], onesb_col[:],
                                    wout_sb[:, k, ds(j * 512, w)],
                                    start=(k == 0), stop=(k == KD - 1))
                n.vector.tensor_scalar_mul(
                    gw128.bitcast(f32)[0:1, ds(j * 512, w)], wsp[:, :w], -1.0)

            enc_sb = setup.tile([T, C], f32, tag="enc")
            n.sync.dma_start(enc_sb[:], enc_d)
            encT = setup.tile([128, KC, T], f32r, tag="encT")
            for c in range(KC):
                tp = sps1.tile([128, T], f32, tag="tp")
                n.tensor.transpose(tp[:], enc_sb[:, ts(c, 128)], ident[:T, :T])
                n.vector.tensor_copy(encT[:, c, :], tp[:])
            ipx_sb = setup.tile([P_IP, C], f32, tag="ipx")
            n.sync.dma_start(ipx_sb[:], ip_d)
            ipT = setup.tile([128, KC, P_IP], f32r, tag="ipT")
            for c in range(KC):
                tp = sps1.tile([128, T], f32, tag="tp")
                n.tensor.transpose(tp[:, :P_IP], ipx_sb[:, ts(c, 128)],
                                   ident[:P_IP, :P_IP])
                n.vector.tensor_copy(ipT[:, c, :], tp[:, :P_IP])

            # ktc (bf16): txt k at cols 0:77, ip k at cols 96:112
            for dt_ in range(KD):
                wk_t = setup_w.tile([128, KC, 128], f32r, tag="wk")
                n.sync.dma_start(
                    wk_t[:],
                    wk_d.rearrange("(co ci) m -> ci co m", ci=128)[:, :, ts(dt_, 128)])
                kp = sps1.tile([128, T], f32, tag="kp")
                for c in range(KC):
                    n.tensor.matmul(kp[:], wk_t[:, c, :].bitcast(f32),
                                    encT[:, c, :].bitcast(f32),
                                    start=(c == 0), stop=(c == KC - 1))
                n.vector.tensor_copy(ktc_sb[:, dt_, 0:T], kp[:])
                wkip_t = setup_w.tile([128, KC, 128], f32r, tag="wk")
                n.sync.dma_start(
                    wkip_t[:],
                    wkip_d.rearrange("(co ci) m -> ci co m", ci=128)[:, :, ts(dt_, 128)])
                kp2 = sps1.tile([128, T], f32, tag="kp")
                for c in range(KC):
                    n.tensor.matmul(kp2[:, :P_IP], wkip_t[:, c, :].bitcast(f32),
                                    ipT[:, c, :].bitcast(f32),
                                    start=(c == 0), stop=(c == KC - 1))
                n.vector.tensor_copy(ktc_sb[:, dt_, IPOFF:IPOFF + P_IP],
                                     kp2[:, :P_IP])

          with tc.tile_pool(name="sps2", bufs=1, space="PSUM") as sps2:
            if True:
                vp = sps2.tile([T, 3 * 512], f32, tag="vp")
                ivp = sps2.tile([128, 3 * 512], f32, tag="ivp")
                for c in range(KC):
                    wv_c = setup_w.tile([128, D], f32r, tag="wv")
                    n.sync.dma_start(
                        wv_c[:],
                        wv_d.rearrange("(co ci) m -> ci co m", ci=128)[:, c, :])
                    wvip_c = setup_w.tile([128, D], f32r, tag="wv")
                    n.sync.dma_start(
                        wvip_c[:],
                        wvip_d.rearrange("(co ci) m -> ci co m", ci=128)[:, c, :])
                    for j in range(3):
                        w = min(512, D - j * 512)
                        n.tensor.matmul(vp[:, ds(j * 512, w)], encT[:, c, :],
                                        wv_c[:, ds(j * 512, w)],
                                        start=(c == 0), stop=(c == KC - 1))
                        n.tensor.matmul(
                            ivp[IPOFF:IPOFF + P_IP, ds(j * 512, w)], ipT[:, c, :],
                            wvip_c[:, ds(j * 512, w)],
                            start=(c == 0), stop=(c == KC - 1),
                            tile_position=(0, IPOFF))
                # scatter [77, 1280] -> vaug [77, 20, 0:64]
                n.vector.tensor_copy(
                    vaug_sb[0:T, :, 0:HD],
                    vp[:, :D].rearrange("p (h c) -> p h c", c=HD))
                n.vector.tensor_copy(
                    ipv_sb[IPOFF:IPOFF + P_IP, :, 0:HD],
                    ivp[IPOFF:IPOFF + P_IP, :D].rearrange("p (h c) -> p h c", c=HD))
                with n.allow_low_precision(reason="v row-sums feed small mean "
                                           "correction; bf16 is plenty"):
                    n.vector.reduce_sum(
                        vaug_sb[0:T, :, HD + 1:HD + 2],
                        vp[:, :D].rearrange("p (h c) -> p h c", c=HD),
                        axis=AXX)
                    n.vector.reduce_sum(
                        ipv_sb[IPOFF:IPOFF + P_IP, :, HD + 1:HD + 2],
                        ivp[IPOFF:IPOFF + P_IP, :D].rearrange(
                            "p (h c) -> p h c", c=HD),
                        axis=AXX)

        # ---------------- main loop over s-blocks --------------------------
        lp = ctx.enter_context(tc.tile_pool(name="lp", bufs=2))
        lp1 = ctx.enter_context(tc.tile_pool(name="lp1", bufs=2))
        lps = ctx.enter_context(tc.tile_pool(name="lps", bufs=2))
        lpp = ctx.enter_context(tc.tile_pool(name="lpp", bufs=12))
        lpo = ctx.enter_context(tc.tile_pool(name="lpo", bufs=2))
        # three disjoint psum rings so consecutive blocks pipeline:
        #  trmm: hs transposes + q-proj (early stages)
        #  scpv: scores + PV (mid stages)
        #  late: gcol transpose + combined transposes + out-proj (late stages)
        ps_a = ctx.enter_context(tc.tile_pool(name="ps_a", bufs=3, space="PSUM"))
        ps_b = ctx.enter_context(tc.tile_pool(name="ps_b", bufs=3, space="PSUM"))
        ps_c = ctx.enter_context(tc.tile_pool(name="ps_c", bufs=2, space="PSUM"))

        PVH = (7, 7, 6)  # heads per 1-bank pv tile
        PVO = (0, 7, 14)
        i32 = mybir.dt.int32

        for b in range(NBLK):
            s0 = b * SB
            # ---- load + transpose hs -> hsT [d, s] (f32) ----
            hsT = lp1.tile([128, KD, SB], f32r, tag="hsT")
            hs_t = {}
            for si in range(2):
                hs_t[si] = lp.tile([128, D], f32, tag=f"hs{si}", name=f"hs{si}")
                n.sync.dma_start(hs_t[si][:], hs_d[ds(s0 + si * 128, 128), :])
            for gi, dp in enumerate(range(0, KD, 2)):
                tp = ps_a.tile([128, 512], f32, tag="trmm")
                for dd in range(2):
                    for si in range(2):
                        n.tensor.transpose(tp.bitcast(f32r)[
                                               :, ds(dd * 256 + si * 128, 128)],
                                           hs_t[si].bitcast(f32r)[
                                               :, ts(dp + dd, 128)],
                                           ident.bitcast(f32r)[:])
                tpv = tp[:].rearrange("p (a b) -> p a b", a=2)
                n.vector.tensor_copy(hsT.bitcast(f32)[:, dp:dp + 2, :], tpv)

            # ---- qT [d, s] (bf16) ----
            qT = lp1.tile([128, KD, SB], bf16, tag="qT")
            for dp in range(0, KD, 2):
                qp = ps_a.tile([128, 512], f32, tag="trmm")
                for dd in range(2):
                    for k in range(KD):
                        n.tensor.matmul(qp[:, ds(dd * SB, SB)],
                                        wq_sb[:, k, ts(dp + dd, 128)], hsT[:, k, :],
                                        start=(k == 0), stop=(k == KD - 1))
                n.vector.tensor_copy(qT[:, dp:dp + 2, :],
                                     qp[:].rearrange("p (a b) -> p a b", a=2))

            # ---- scores + exp per head-pair: [112, 512] ----
            pTc = {}
            for hp in range(KD):
                sc = ps_b.tile([128, 512], f32, tag="scpv", name=f"sc{hp}")
                for half in range(2):
                    n.tensor.matmul(sc[0:CAT, ds(half * SB, SB)],
                                    ktc_sb[ds(64 * half, 64), hp, :],
                                    qT[ds(64 * half, 64), hp, :],
                                    start=True, stop=True)
                pTc[hp] = lpp.tile([CAT, 512], bf16, tag="pTc", name=f"pTc{hp}")
                n.scalar.activation(pTc[hp][:], sc[0:CAT, :], FT.Exp, scale=SCALE)

            # ---- PV + normalize ----
            lat_n = lp1.tile([128, 2, H, HD], bf16, tag="lat_n")
            ipo_n = lp1.tile([128, 2, H, HD], bf16, tag="ipo_n")
            sm = lps.tile([128, 2, 2, H, 2], f32, tag="sm")
            recip = lps.tile([128, 2, 2, H], f32, tag="recip")
            msum = lps.tile([128, 2, 2, H], f32, tag="msum")
            st = lps.tile([128, 2, 24], f32, tag="st")
            sq_scr = lps.tile([128, H, HD], bf16, tag="sqscr")

            for si in range(2):
                for br in range(2):  # 0 = txt, 1 = ip
                    dest = lat_n if br == 0 else ipo_n
                    for g in range(3):
                        nh, h0 = PVH[g], PVO[g]
                        pv = ps_b.tile([128, 512], f32, tag="scpv",
                                       name=f"pv{g}")
                        for j in range(nh):
                            h = h0 + j
                            hp, half = h // 2, h % 2
                            col0 = half * SB + si * 128
                            if br == 0:
                                lhsT = pTc[hp][0:T, ds(col0, 128)]
                                rhs = vaug_sb[0:T, h, :]
                            else:
                                lhsT = pTc[hp][IPOFF:IPOFF + P_IP, ds(col0, 128)]
                                rhs = ipv_sb[IPOFF:IPOFF + P_IP, h, :]
                            n.tensor.matmul(pv[:, ds(j * VW, VW)],
                                            lhsT, rhs, start=True, stop=True,
                                            tile_position=(
                                                (IPOFF, 0) if br else (0, 0)))
                        pvj = pv[:, 0:nh * VW].rearrange("p (j c) -> p j c", c=VW)
                        n.vector.tensor_copy(sm[:, si, br, ds(h0, nh), :],
                                             pvj[:, :, HD:HD + 2])
                        n.vector.reciprocal(recip[:, si, br, ds(h0, nh)],
                                            sm[:, si, br, ds(h0, nh), 0])
                        n.vector.tensor_mul(msum[:, si, br, ds(h0, nh)],
                                            sm[:, si, br, ds(h0, nh), 1],
                                            recip[:, si, br, ds(h0, nh)])
                        n.vector.tensor_tensor(
                            dest[:, si, ds(h0, nh), :],
                            pvj[:, :, 0:HD],
                            recip[:, si, br, ds(h0, nh), None].to_broadcast(
                                [128, nh, HD]),
                            op=ALU.mult)

            # ---- norm_ipa stats ----
            for si in range(2):
                n.vector.reduce_sum(st[:, si, 0:1], msum[:, si, 0, :], axis=AXX)
                n.vector.reduce_sum(st[:, si, 1:2], msum[:, si, 1, :], axis=AXX)
                n.vector.tensor_tensor_reduce(
                    out=sq_scr[:], in0=lat_n[:, si], in1=lat_n[:, si],
                    scale=1.0, scalar=0.0, op0=ALU.mult, op1=ALU.add,
                    accum_out=st[:, si, 2:3])
                n.vector.tensor_tensor_reduce(
                    out=sq_scr[:], in0=ipo_n[:, si], in1=ipo_n[:, si],
                    scale=1.0, scalar=0.0, op0=ALU.mult, op1=ALU.add,
                    accum_out=st[:, si, 3:4])
                # means and variances
                n.vector.tensor_scalar_mul(st[:, si, 4:5], st[:, si, 0:1], 1.0 / D)
                n.vector.tensor_scalar_mul(st[:, si, 5:6], st[:, si, 1:2], 1.0 / D)
                n.vector.tensor_mul(st[:, si, 6:7], st[:, si, 4:5], st[:, si, 4:5])
                n.vector.tensor_mul(st[:, si, 7:8], st[:, si, 5:6], st[:, si, 5:6])
                n.vector.tensor_scalar(out=st[:, si, 8:9], in0=st[:, si, 2:3],
                                       scalar1=1.0 / D, scalar2=st[:, si, 6:7],
                                       op0=ALU.mult, op1=ALU.subtract)
                n.vector.tensor_scalar(out=st[:, si, 9:10], in0=st[:, si, 3:4],
                                       scalar1=1.0 / D, scalar2=st[:, si, 7:8],
                                       op0=ALU.mult, op1=ALU.subtract)
            # std = sqrt(var) via DVE: fast-inverse-sqrt init + 3 Newton
            # iters (keeps ACT on the Exp table all kernel long)
            vv = st[:, :, 8:10]
            yy = st[:, :, 10:12]
            t0 = st[:, :, 12:14]
            n.vector.tensor_scalar(out=yy.bitcast(i32), in0=vv.bitcast(i32),
                                   scalar1=1, scalar2=None,
                                   op0=ALU.logical_shift_right)
            n.vector.tensor_scalar(out=yy.bitcast(i32), in0=yy.bitcast(i32),
                                   scalar1=-1, scalar2=0x5f3759df,
                                   op0=ALU.mult, op1=ALU.add)
            for _ in range(3):
                n.vector.tensor_mul(t0[:], yy[:], yy[:])
                n.vector.tensor_mul(t0[:], t0[:], vv[:])
                n.vector.tensor_scalar(out=t0[:], in0=t0[:], scalar1=-0.5,
                                       scalar2=1.5, op0=ALU.mult, op1=ALU.add)
                n.vector.tensor_mul(yy[:], yy[:], t0[:])
            # y ~= rsqrt(var); std = var * y
            n.vector.tensor_mul(yy[:], vv[:], yy[:])
            hsT2 = lp1.tile([128, KD, SB], bf16, tag="hsT2")
            gp = lps.tile([128, 2, 128], f32r, tag="gp")
            for si in range(2):
                n.vector.tensor_scalar_add(st[:, si, 12:13], st[:, si, 11:12], EPS)
                n.vector.reciprocal(st[:, si, 13:14], st[:, si, 12:13])
                n.vector.tensor_mul(st[:, si, 14:15], st[:, si, 10:11],
                                    st[:, si, 13:14])
                # gneg = alpha*mean_ip - mean_lat; gcol = [gneg, 1.0]
                n.vector.scalar_tensor_tensor(
                    out=st[:, si, 15:16], in0=st[:, si, 5:6],
                    scalar=st[:, si, 14:15], in1=st[:, si, 4:5],
                    op0=ALU.mult, op1=ALU.subtract)
                n.vector.memset(st[:, si, 16:17], 1.0)
                gt = ps_c.tile([128, 512], f32, tag="late", name="gt")
                n.tensor.transpose(gt[0:2, 0:128], st[:, si, 15:17], ident[:])
                n.vector.tensor_copy(gp.bitcast(f32)[0:2, si, :], gt[0:2, 0:128])
                # combine in place: lat_n <- lat_n + alpha * ipo_n
                n.gpsimd.scalar_tensor_tensor(
                    out=lat_n[:, si].rearrange("p h c -> p (h c)"),
                    in0=ipo_n[:, si].rearrange("p h c -> p (h c)"),
                    scalar=st[:, si, 14:15],
                    in1=lat_n[:, si].rearrange("p h c -> p (h c)"),
                    op0=ALU.mult, op1=ALU.add)

            # ---- transpose combined -> hsT2 [d, s] (bf16) ----
            for gi, dp in enumerate(range(0, KD, 2)):
                tp = ps_c.tile([128, 512], bf16, tag="late", name="tp2")
                for dd in range(2):
                    for si in range(2):
                        n.tensor.transpose(
                            tp[:, ds(dd * 256 + si * 128, 128)],
                            lat_n[:, si].rearrange(
                                "p h c -> p (h c)")[:, ts(dp + dd, 128)],
                            identb[:])
                n.vector.tensor_copy(hsT2[:, dp:dp + 2, :],
                                     tp[:].rearrange("p (a b) -> p a b", a=2))

            # ---- out projection (+ bias - g*wsum via f32r matmul) ----
            for si in range(2):
                for j in range(3):
                    w = min(512, D - j * 512)
                    op = ps_c.tile([128, 512], f32, tag="late", name="op")
                    n.tensor.matmul(op[:, :w], gp[:, si, :],
                                    gw128[:, ds(j * 512, w)],
                                    start=True, stop=False)
                    for k in range(KD):
                        n.tensor.matmul(op[:, :w], hsT2[:, k, ts(si, 128)],
                                        wout_sb[:, k, ds(j * 512, w)],
                                        start=False, stop=(k == KD - 1))
                    ost = lpo.tile([128, 512], f32, tag="ost")
                    n.scalar.copy(ost[:, :w], op[:, :w])
                    n.sync.dma_start(
                        out_d[ds(s0 + si * 128, 128), ds(j * 512, w)], ost[:, :w])
    nc.compile()
    return nc


def _get_nc():
    if "nc" not in _CACHE:
        _CACHE["nc"] = _build()
    return _CACHE["nc"]


def kernel(**inputs) -> np.ndarray:
    nc = _get_nc()
    f = lambda x: np.ascontiguousarray(np.asarray(x), dtype=np.float32)
    shared = {k: f(inputs[k]) for k in
              ("w_q", "w_k", "w_v", "w_k_ip", "w_v_ip", "w_out", "b_out")}
    hs = f(inputs["hidden_states"])
    enc = f(inputs["encoder_hidden_states"])
    ipx = f(inputs["ip_hidden_states"])
    in_maps = [
        dict(shared, hidden_states=hs[i], encoder_hidden_states=enc[i],
             ip_hidden_states=ipx[i])
        for i in range(8)
    ]
    res = bass_utils.run_bass_kernel_spmd(nc, in_maps, core_ids=list(range(8)))
    return np.stack([res.results[i]["out"] for i in range(8)], axis=0)


if __name__ == "__main__":
    rng = np.random.default_rng(0)
    ins = {
        "hidden_states": rng.standard_normal((B, S, D), dtype=np.float32),
        "encoder_hidden_states": rng.standard_normal((B, T, C), dtype=np.float32),
        "ip_hidden_states": rng.standard_normal((B, P_IP, C), dtype=np.float32),
        "w_q": (rng.standard_normal((D, D), dtype=np.float32) * 0.02),
        "w_k": (rng.standard_normal((C, D), dtype=np.float32) * 0.02),
        "w_v": (rng.standard_normal((C, D), dtype=np.float32) * 0.02),
        "w_k_ip": (rng.standard_normal((C, D), dtype=np.float32) * 0.02),
        "w_v_ip": (rng.standard_normal((C, D), dtype=np.float32) * 0.02),
        "w_out": (rng.standard_normal((D, D), dtype=np.float32) * 0.02),
        "b_out": np.zeros((D,), dtype=np.float32),
    }
    out = kernel(**ins)
    print("out", out.shape, out.dtype, float(np.abs(out).max()))


# revision 14
# speedup vs baseline: 1.0016x; 1.0016x over previous
"""Bass/Tile kernel for nn_CustomCrossAttnProcessor (8-core data-parallel).

Each NeuronCore processes one batch element (B=8 == n_cores).
Per-core compute, one batch element:
  q = hs @ w_q                       (f32r matmuls, N=256)
  k/v = enc @ w_{k,v}; ip_k/ip_v = ip @ w_{k,v}_ip   (setup)
  merged scores [112, 512] per head-pair: txt rows 0:77, ip rows 96:112,
  one exp per pair (no max-subtract: |scores*scale| <= ~5), probs bf16
  PV with ones/vrowsum columns appended to v -> softmax denominators and
  row-means for free; all 20 heads of one (si, branch) live in two
  2-bank psum tiles -> one big normalize op per group
  norm_ipa stats via tensor_tensor_reduce; -g*wsum + bias folded into the
  out-projection as an extra f32r matmul; out = hs_sum @ w_out (bf16).
"""
import sys

for _p in ("/opt/trn_rl_repo",):
    if _p not in sys.path:
        sys.path.append(_p)

from contextlib import ExitStack

import numpy as np

import concourse.bass as bass  # noqa: F401
import concourse.tile as tile
import concourse.mybir as mybir
from concourse import bass_utils, bacc
from concourse.bass import ts, ds
from concourse.masks import make_identity

B, S, D = 8, 4096, 1280
T, P_IP, C = 77, 16, 2048
H, HD = 20, 64
SB = 256            # tokens per s-block
NBLK = S // SB      # 16
SCALE = HD ** -0.5  # 0.125
EPS = 1e-7
KD = D // 128       # 10
KC = C // 128       # 16
CAT = 112           # score rows: txt [0:77], zero pad [77:96], ip [96:112]
IPOFF = 96
VW = HD + 2         # 66: v cols + ones col (softmax sum) + v-rowsum col
ALU = mybir.AluOpType
FT = mybir.ActivationFunctionType
AXX = mybir.AxisListType.X

f32 = mybir.dt.float32
f32r = mybir.dt.float32r
bf16 = mybir.dt.bfloat16

_CACHE = {}


def _build():
    nc = bacc.Bacc(
        "TRN2", target_bir_lowering=False, debug=False, enable_asserts=False,
        num_devices=8,
    )
    hs_d = nc.dram_tensor("hidden_states", [S, D], f32, kind="ExternalInput").ap()
    enc_d = nc.dram_tensor("encoder_hidden_states", [T, C], f32,
                           kind="ExternalInput").ap()
    ip_d = nc.dram_tensor("ip_hidden_states", [P_IP, C], f32,
                          kind="ExternalInput").ap()
    wq_d = nc.dram_tensor("w_q", [D, D], f32r, kind="ExternalInput").ap()
    wk_d = nc.dram_tensor("w_k", [C, D], f32r, kind="ExternalInput").ap()
    wv_d = nc.dram_tensor("w_v", [C, D], f32r, kind="ExternalInput").ap()
    wkip_d = nc.dram_tensor("w_k_ip", [C, D], f32r, kind="ExternalInput").ap()
    wvip_d = nc.dram_tensor("w_v_ip", [C, D], f32r, kind="ExternalInput").ap()
    wout_d = nc.dram_tensor("w_out", [D, D], f32, kind="ExternalInput").ap()
    bout_d = nc.dram_tensor("b_out", [D], f32, kind="ExternalInput").ap()
    out_d = nc.dram_tensor("out", [S, D], f32, kind="ExternalOutput").ap()

    with tile.TileContext(nc) as tc, ExitStack() as ctx:
        n = tc.nc
        const = ctx.enter_context(tc.tile_pool(name="const", bufs=1))
        wq_sb = const.tile([128, KD, D], f32r)
        wout_sb = const.tile([128, KD, D], bf16)
        ktc_sb = const.tile([128, KD, CAT], bf16)
        vaug_sb = const.tile([128, H, VW], bf16)   # rows 0:77 hold txt v
        ipv_sb = const.tile([128, H, VW], bf16)    # rows 96:112 hold ip v
        ident = const.tile([128, 128], f32)
        identb = const.tile([128, 128], bf16)
        onesb_col = const.tile([128, 1], bf16)
        b_row = const.tile([1, D], f32)
        gw128 = const.tile([128, D], f32r)  # row0=-colsum(w_out), row1=b_out

        make_identity(n, ident[:])
        make_identity(n, identb[:])
        n.vector.memset(onesb_col[:], 1.0)
        n.vector.memset(ktc_sb[:, :, T:IPOFF], 0.0)
        n.vector.memset(vaug_sb[0:T, :, HD:HD + 1], 1.0)
        n.vector.memset(ipv_sb[IPOFF:IPOFF + P_IP, :, HD:HD + 1], 1.0)
        n.vector.memset(gw128.bitcast(f32)[:], 0.0)
        n.sync.dma_start(wq_sb[:], wq_d.rearrange("(ko ki) m -> ki ko m", ki=128))
        n.sync.dma_start(b_row[:], bout_d[None, :])
        n.vector.tensor_copy(gw128.bitcast(f32)[1:2, :], b_row[:])

        # ---------------- setup: weights, k/v projections -------------------
        with tc.tile_pool(name="setup", bufs=1) as setup, \
             tc.tile_pool(name="setup_w", bufs=3) as setup_w:
          with tc.tile_pool(name="sps1", bufs=2, space="PSUM") as sps1:
            # wout: load f32 (two halves), cast to bf16
            for half in range(2):
                wout_f = setup.tile([128, KD // 2, D], f32, tag="woutf")
                n.sync.dma_start(
                    wout_f[:],
                    wout_d.rearrange("(ko ki) m -> ki ko m", ki=128)[
                        :, ds(half * (KD // 2), KD // 2), :])
                for k in range(KD // 2):
                    n.scalar.copy(wout_sb[:, half * (KD // 2) + k, :],
                                  wout_f[:, k, :])
            # column sums of w_out for the -g*wsum fold
            for j in range(3):
                w = min(512, D - j * 512)
                wsp = sps1.tile([1, 512], f32, tag="wsp", bufs=1)
                for k in range(KD):
                    n.tensor.matmul(wsp[:, :w], onesb_col[:],
                                    wout_sb[:, k, ds(j * 512, w)],
                                    start=(k == 0), stop=(k == KD - 1))
                n.vector.tensor_scalar_mul(
                    gw128.bitcast(f32)[0:1, ds(j * 512, w)], wsp[:, :w], -1.0)

            enc_sb = setup.tile([T, C], f32, tag="enc")
            n.sync.dma_start(enc_sb[:], enc_d)
            encT = setup.tile([128, KC, T], f32r, tag="encT")
            for c in range(KC):
                tp = sps1.tile([128, T], f32, tag="tp")
                n.tensor.transpose(tp[:], enc_sb[:, ts(c, 128)], ident[:T, :T])
                n.vector.tensor_copy(encT[:, c, :], tp[:])
            ipx_sb = setup.tile([P_IP, C], f32, tag="ipx")
            n.sync.dma_start(ipx_sb[:], ip_d)
            ipT = setup.tile([128, KC, P_IP], f32r, tag="ipT")
            for c in range(KC):
                tp = sps1.tile([128, T], f32, tag="tp")
                n.tensor.transpose(tp[:, :P_IP], ipx_sb[:, ts(c, 128)],
                                   ident[:P_IP, :P_IP])
                n.vector.tensor_copy(ipT[:, c, :], tp[:, :P_IP])

            # ktc (bf16): txt k at cols 0:77, ip k at cols 96:112
            for dt_ in range(KD):
                wk_t = setup_w.tile([128, KC, 128], f32r, tag="wk")
                n.sync.dma_start(
                    wk_t[:],
                    wk_d.rearrange("(co ci) m -> ci co m", ci=128)[:, :, ts(dt_, 128)])
                kp = sps1.tile([128, T], f32, tag="kp")
                for c in range(KC):
                    n.tensor.matmul(kp[:], wk_t[:, c, :].bitcast(f32),
                                    encT[:, c, :].bitcast(f32),
                                    start=(c == 0), stop=(c == KC - 1))
                n.vector.tensor_copy(ktc_sb[:, dt_, 0:T], kp[:])
                wkip_t = setup_w.tile([128, KC, 128], f32r, tag="wk")
                n.sync.dma_start(
                    wkip_t[:],
                    wkip_d.rearrange("(co ci) m -> ci co m", ci=128)[:, :, ts(dt_, 128)])
                kp2 = sps1.tile([128, T], f32, tag="kp")
                for c in range(KC):
                    n.tensor.matmul(kp2[:, :P_IP], wkip_t[:, c, :].bitcast(f32),
                                    ipT[:, c, :].bitcast(f32),
                                    start=(c == 0), stop=(c == KC - 1))
                n.vector.tensor_copy(ktc_sb[:, dt_, IPOFF:IPOFF + P_IP],
                                     kp2[:, :P_IP])

          with tc.tile_pool(name="sps2", bufs=1, space="PSUM") as sps2:
            if True:
                vp = sps2.tile([T, 3 * 512], f32, tag="vp")
                ivp = sps2.tile([128, 3 * 512], f32, tag="ivp")
                for c in range(KC):
                    wv_c = setup_w.tile([128, D], f32r, tag="wv")
                    n.sync.dma_start(
                        wv_c[:],
                        wv_d.rearrange("(co ci) m -> ci co m", ci=128)[:, c, :])
                    wvip_c = setup_w.tile([128, D], f32r, tag="wv")
                    n.sync.dma_start(
                        wvip_c[:],
                        wvip_d.rearrange("(co ci) m -> ci co m", ci=128)[:, c, :])
                    for j in range(3):
                        w = min(512, D - j * 512)
                        n.tensor.matmul(vp[:, ds(j * 512, w)], encT[:, c, :],
                                        wv_c[:, ds(j * 512, w)],
                                        start=(c == 0), stop=(c == KC - 1))
                        n.tensor.matmul(
                            ivp[IPOFF:IPOFF + P_IP, ds(j * 512, w)], ipT[:, c, :],
                            wvip_c[:, ds(j * 512, w)],
                            start=(c == 0), stop=(c == KC - 1),
                            tile_position=(0, IPOFF))
                # scatter [77, 1280] -> vaug [77, 20, 0:64]
                n.vector.tensor_copy(
                    vaug_sb[0:T, :, 0:HD],
                    vp[:, :D].rearrange("p (h c) -> p h c", c=HD))
                n.vector.tensor_copy(
                    ipv_sb[IPOFF:IPOFF + P_IP, :, 0:HD],
                    ivp[IPOFF:IPOFF + P_IP, :D].rearrange("p (h c) -> p h c", c=HD))
                with n.allow_low_precision(reason="v row-sums feed small mean "
                                           "correction; bf16 is plenty"):
                    n.vector.reduce_sum(
                        vaug_sb[0:T, :, HD + 1:HD + 2],
                        vp[:, :D].rearrange("p (h c) -> p h c", c=HD),
                        axis=AXX)
                    n.vector.reduce_sum(
                        ipv_sb[IPOFF:IPOFF + P_IP, :, HD + 1:HD + 2],
                        ivp[IPOFF:IPOFF + P_IP, :D].rearrange(
                            "p (h c) -> p h c", c=HD),
                        axis=AXX)

        # ---------------- main loop over s-blocks --------------------------
        lp = ctx.enter_context(tc.tile_pool(name="lp", bufs=2))
        lp1 = ctx.enter_context(tc.tile_pool(name="lp1", bufs=2))
        lps = ctx.enter_context(tc.tile_pool(name="lps", bufs=2))
        lpp = ctx.enter_context(tc.tile_pool(name="lpp", bufs=12))
        lpo = ctx.enter_context(tc.tile_pool(name="lpo", bufs=2))
        # three disjoint psum rings so consecutive blocks pipeline:
        #  trmm: hs transposes + q-proj (early stages)
        #  scpv: scores + PV (mid stages)
        #  late: gcol transpose + combined transposes + out-proj (late stages)
        ps_a = ctx.enter_context(tc.tile_pool(name="ps_a", bufs=3, space="PSUM"))
        ps_b = ctx.enter_context(tc.tile_pool(name="ps_b", bufs=3, space="PSUM"))
        ps_c = ctx.enter_context(tc.tile_pool(name="ps_c", bufs=2, space="PSUM"))

        PVH = (7, 7, 6)  # heads per 1-bank pv tile
        PVO = (0, 7, 14)
        i32 = mybir.dt.int32

        for b in range(NBLK):
            s0 = b * SB
            # ---- load + transpose hs -> hsT [d, s] (f32) ----
            hsT = lp1.tile([128, KD, SB], f32r, tag="hsT")
            hs_t = {}
            for si in range(2):
                hs_t[si] = lp.tile([128, D], f32, tag=f"hs{si}", name=f"hs{si}")
                n.sync.dma_start(hs_t[si][:], hs_d[ds(s0 + si * 128, 128), :])
            for gi, dp in enumerate(range(0, KD, 2)):
                tp = ps_a.tile([128, 512], f32, tag="trmm")
                for dd in range(2):
                    for si in range(2):
                        n.tensor.transpose(tp.bitcast(f32r)[
                                               :, ds(dd * 256 + si * 128, 128)],
                                           hs_t[si].bitcast(f32r)[
                                               :, ts(dp + dd, 128)],
                                           ident.bitcast(f32r)[:])
                tpv = tp[:].rearrange("p (a b) -> p a b", a=2)
                n.vector.tensor_copy(hsT.bitcast(f32)[:, dp:dp + 2, :], tpv)

            # ---- qT [d, s] (bf16) ----
            qT = lp1.tile([128, KD, SB], bf16, tag="qT")
            for dp in range(0, KD, 2):
                qp = ps_a.tile([128, 512], f32, tag="trmm")
                for dd in range(2):
                    for k in range(KD):
                        n.tensor.matmul(qp[:, ds(dd * SB, SB)],
                                        wq_sb[:, k, ts(dp + dd, 128)], hsT[:, k, :],
                                        start=(k == 0), stop=(k == KD - 1))
                n.scalar.copy(qT[:, dp:dp + 2, :],
                              qp[:].rearrange("p (a b) -> p a b", a=2))

            # ---- scores + exp per head-pair: [112, 512] ----
            pTc = {}
            for hp in range(KD):
                sc = ps_b.tile([128, 512], f32, tag="scpv", name=f"sc{hp}")
                for half in range(2):
                    n.tensor.matmul(sc[0:CAT, ds(half * SB, SB)],
                                    ktc_sb[ds(64 * half, 64), hp, :],
                                    qT[ds(64 * half, 64), hp, :],
                                    start=True, stop=True)
                pTc[hp] = lpp.tile([CAT, 512], bf16, tag="pTc", name=f"pTc{hp}")
                n.scalar.activation(pTc[hp][:], sc[0:CAT, :], FT.Exp, scale=SCALE)

            # ---- PV + normalize ----
            lat_n = lp1.tile([128, 2, H, HD], bf16, tag="lat_n")
            ipo_n = lp1.tile([128, 2, H, HD], bf16, tag="ipo_n")
            sm = lps.tile([128, 2, 2, H, 2], f32, tag="sm")
            recip = lps.tile([128, 2, 2, H], f32, tag="recip")
            msum = lps.tile([128, 2, 2, H], f32, tag="msum")
            st = lps.tile([128, 2, 24], f32, tag="st")
            sq_scr = lps.tile([128, H, HD], bf16, tag="sqscr")

            for si in range(2):
                for br in range(2):  # 0 = txt, 1 = ip
                    dest = lat_n if br == 0 else ipo_n
                    for g in range(3):
                        nh, h0 = PVH[g], PVO[g]
                        pv = ps_b.tile([128, 512], f32, tag="scpv",
                                       name=f"pv{g}")
                        for j in range(nh):
                            h = h0 + j
                            hp, half = h // 2, h % 2
                            col0 = half * SB + si * 128
                            if br == 0:
                                lhsT = pTc[hp][0:T, ds(col0, 128)]
                                rhs = vaug_sb[0:T, h, :]
                            else:
                                lhsT = pTc[hp][IPOFF:IPOFF + P_IP, ds(col0, 128)]
                                rhs = ipv_sb[IPOFF:IPOFF + P_IP, h, :]
                            n.tensor.matmul(pv[:, ds(j * VW, VW)],
                                            lhsT, rhs, start=True, stop=True,
                                            tile_position=(
                                                (IPOFF, 0) if br else (0, 0)))
                        pvj = pv[:, 0:nh * VW].rearrange("p (j c) -> p j c", c=VW)
                        n.vector.tensor_copy(sm[:, si, br, ds(h0, nh), :],
                                             pvj[:, :, HD:HD + 2])
                        n.vector.reciprocal(recip[:, si, br, ds(h0, nh)],
                                            sm[:, si, br, ds(h0, nh), 0])
                        n.vector.tensor_mul(msum[:, si, br, ds(h0, nh)],
                                            sm[:, si, br, ds(h0, nh), 1],
                                            recip[:, si, br, ds(h0, nh)])
                        n.vector.tensor_tensor(
                            dest[:, si, ds(h0, nh), :],
                            pvj[:, :, 0:HD],
                            recip[:, si, br, ds(h0, nh), None].to_broadcast(
                                [128, nh, HD]),
                            op=ALU.mult)

            # ---- norm_ipa stats ----
            for si in range(2):
                n.vector.reduce_sum(st[:, si, 0:1], msum[:, si, 0, :], axis=AXX)
                n.vector.reduce_sum(st[:, si, 1:2], msum[:, si, 1, :], axis=AXX)
                n.vector.tensor_tensor_reduce(
                    out=sq_scr[:], in0=lat_n[:, si], in1=lat_n[:, si],
                    scale=1.0, scalar=0.0, op0=ALU.mult, op1=ALU.add,
                    accum_out=st[:, si, 2:3])
                n.vector.tensor_tensor_reduce(
                    out=sq_scr[:], in0=ipo_n[:, si], in1=ipo_n[:, si],
                    scale=1.0, scalar=0.0, op0=ALU.mult, op1=ALU.add,
                    accum_out=st[:, si, 3:4])
                # means and variances
                n.vector.tensor_scalar_mul(st[:, si, 4:5], st[:, si, 0:1], 1.0 / D)
                n.vector.tensor_scalar_mul(st[:, si, 5:6], st[:, si, 1:2], 1.0 / D)
                n.vector.tensor_mul(st[:, si, 6:7], st[:, si, 4:5], st[:, si, 4:5])
                n.vector.tensor_mul(st[:, si, 7:8], st[:, si, 5:6], st[:, si, 5:6])
                n.vector.tensor_scalar(out=st[:, si, 8:9], in0=st[:, si, 2:3],
                                       scalar1=1.0 / D, scalar2=st[:, si, 6:7],
                                       op0=ALU.mult, op1=ALU.subtract)
                n.vector.tensor_scalar(out=st[:, si, 9:10], in0=st[:, si, 3:4],
                                       scalar1=1.0 / D, scalar2=st[:, si, 7:8],
                                       op0=ALU.mult, op1=ALU.subtract)
            # std = sqrt(var) via DVE: fast-inverse-sqrt init + 3 Newton
            # iters (keeps ACT on the Exp table all kernel long)
            vv = st[:, :, 8:10]
            yy = st[:, :, 10:12]
            t0 = st[:, :, 12:14]
            n.vector.tensor_scalar(out=yy.bitcast(i32), in0=vv.bitcast(i32),
                                   scalar1=1, scalar2=None,
                                   op0=ALU.logical_shift_right)
            n.vector.tensor_scalar(out=yy.bitcast(i32), in0=yy.bitcast(i32),
                                   scalar1=-1, scalar2=0x5f3759df,
                                   op0=ALU.mult, op1=ALU.add)
            for _ in range(3):
                n.vector.tensor_mul(t0[:], yy[:], yy[:])
                n.vector.tensor_mul(t0[:], t0[:], vv[:])
                n.vector.tensor_scalar(out=t0[:], in0=t0[:], scalar1=-0.5,
                                       scalar2=1.5, op0=ALU.mult, op1=ALU.add)
                n.vector.tensor_mul(yy[:], yy[:], t0[:])
            # y ~= rsqrt(var); std = var * y
            n.vector.tensor_mul(yy[:], vv[:], yy[:])
            hsT2 = lp1.tile([128, KD, SB], bf16, tag="hsT2")
            gp = lps.tile([128, 2, 128], f32r, tag="gp")
            for si in range(2):
                n.vector.tensor_scalar_add(st[:, si, 12:13], st[:, si, 11:12], EPS)
                n.vector.reciprocal(st[:, si, 13:14], st[:, si, 12:13])
                n.vector.tensor_mul(st[:, si, 14:15], st[:, si, 10:11],
                                    st[:, si, 13:14])
                # gneg = alpha*mean_ip - mean_lat; gcol = [gneg, 1.0]
                n.vector.scalar_tensor_tensor(
                    out=st[:, si, 15:16], in0=st[:, si, 5:6],
                    scalar=st[:, si, 14:15], in1=st[:, si, 4:5],
                    op0=ALU.mult, op1=ALU.subtract)
                n.vector.memset(st[:, si, 16:17], 1.0)
                gt = ps_c.tile([128, 512], f32, tag="late", name="gt")
                n.tensor.transpose(gt[0:2, 0:128], st[:, si, 15:17], ident[:])
                n.vector.tensor_copy(gp.bitcast(f32)[0:2, si, :], gt[0:2, 0:128])
                # combine in place: lat_n <- lat_n + alpha * ipo_n
                n.gpsimd.scalar_tensor_tensor(
                    out=lat_n[:, si].rearrange("p h c -> p (h c)"),
                    in0=ipo_n[:, si].rearrange("p h c -> p (h c)"),
                    scalar=st[:, si, 14:15],
                    in1=lat_n[:, si].rearrange("p h c -> p (h c)"),
                    op0=ALU.mult, op1=ALU.add)

            # ---- transpose combined -> hsT2 [d, s] (bf16) ----
            for gi, dp in enumerate(range(0, KD, 2)):
                tp = ps_c.tile([128, 512], bf16, tag="late", name="tp2")
                for dd in range(2):
                    for si in range(2):
                        n.tensor.transpose(
                            tp[:, ds(dd * 256 + si * 128, 128)],
                            lat_n[:, si].rearrange(
                                "p h c -> p (h c)")[:, ts(dp + dd, 128)],
                            identb[:])
                n.vector.tensor_copy(hsT2[:, dp:dp + 2, :],
                                     tp[:].rearrange("p (a b) -> p a b", a=2))

            # ---- out projection (+ bias - g*wsum via f32r matmul) ----
            for si in range(2):
                for j in range(3):
                    w = min(512, D - j * 512)
                    op = ps_c.tile([128, 512], f32, tag="late", name="op")
                    n.tensor.matmul(op[:, :w], gp[:, si, :],
                                    gw128[:, ds(j * 512, w)],
                                    start=True, stop=False)
                    for k in range(KD):
                        n.tensor.matmul(op[:, :w], hsT2[:, k, ts(si, 128)],
                                        wout_sb[:, k, ds(j * 512, w)],
                                        start=False, stop=(k == KD - 1))
                    ost = lpo.tile([128, 512], f32, tag="ost")
                    n.scalar.copy(ost[:, :w], op[:, :w])
                    n.sync.dma_start(
                        out_d[ds(s0 + si * 128, 128), ds(j * 512, w)], ost[:, :w])
    nc.compile()
    return nc


def _get_nc():
    if "nc" not in _CACHE:
        _CACHE["nc"] = _build()
    return _CACHE["nc"]


def kernel(**inputs) -> np.ndarray:
    nc = _get_nc()
    f = lambda x: np.ascontiguousarray(np.asarray(x), dtype=np.float32)
    shared = {k: f(inputs[k]) for k in
              ("w_q", "w_k", "w_v", "w_k_ip", "w_v_ip", "w_out", "b_out")}
    hs = f(inputs["hidden_states"])
    enc = f(inputs["encoder_hidden_states"])
    ipx = f(inputs["ip_hidden_states"])
    in_maps = [
        dict(shared, hidden_states=hs[i], encoder_hidden_states=enc[i],
             ip_hidden_states=ipx[i])
        for i in range(8)
    ]
    res = bass_utils.run_bass_kernel_spmd(nc, in_maps, core_ids=list(range(8)))
    return np.stack([res.results[i]["out"] for i in range(8)], axis=0)


if __name__ == "__main__":
    rng = np.random.default_rng(0)
    ins = {
        "hidden_states": rng.standard_normal((B, S, D), dtype=np.float32),
        "encoder_hidden_states": rng.standard_normal((B, T, C), dtype=np.float32),
        "ip_hidden_states": rng.standard_normal((B, P_IP, C), dtype=np.float32),
        "w_q": (rng.standard_normal((D, D), dtype=np.float32) * 0.02),
        "w_k": (rng.standard_normal((C, D), dtype=np.float32) * 0.02),
        "w_v": (rng.standard_normal((C, D), dtype=np.float32) * 0.02),
        "w_k_ip": (rng.standard_normal((C, D), dtype=np.float32) * 0.02),
        "w_v_ip": (rng.standard_normal((C, D), dtype=np.float32) * 0.02),
        "w_out": (rng.standard_normal((D, D), dtype=np.float32) * 0.02),
        "b_out": np.zeros((D,), dtype=np.float32),
    }
    out = kernel(**ins)
    print("out", out.shape, out.dtype, float(np.abs(out).max()))


# revision 15
# speedup vs baseline: 1.0690x; 1.0673x over previous
"""Bass/Tile kernel for nn_CustomCrossAttnProcessor (8-core data-parallel).

Each NeuronCore processes one batch element (B=8 == n_cores).
Per-core compute, one batch element:
  q = hs @ w_q                       (f32r matmuls, N=256)
  k/v = enc @ w_{k,v}; ip_k/ip_v = ip @ w_{k,v}_ip   (setup)
  merged scores [112, 512] per head-pair: txt rows 0:77, ip rows 96:112,
  one exp per pair (no max-subtract: |scores*scale| <= ~5), probs bf16
  PV with ones/vrowsum columns appended to v -> softmax denominators and
  row-means for free; all 20 heads of one (si, branch) live in two
  2-bank psum tiles -> one big normalize op per group
  norm_ipa stats via tensor_tensor_reduce; -g*wsum + bias folded into the
  out-projection as an extra f32r matmul; out = hs_sum @ w_out (bf16).
"""
import sys

for _p in ("/opt/trn_rl_repo",):
    if _p not in sys.path:
        sys.path.append(_p)

from contextlib import ExitStack

import numpy as np

import concourse.bass as bass  # noqa: F401
import concourse.tile as tile
import concourse.mybir as mybir
from concourse import bass_utils, bacc
from concourse.bass import ts, ds
from concourse.masks import make_identity

B, S, D = 8, 4096, 1280
T, P_IP, C = 77, 16, 2048
H, HD = 20, 64
SB = 256            # tokens per s-block
NBLK = S // SB      # 16
SCALE = HD ** -0.5  # 0.125
EPS = 1e-7
KD = D // 128       # 10
KC = C // 128       # 16
CAT = 112           # score rows: txt [0:77], zero pad [77:96], ip [96:112]
IPOFF = 96
VW = HD + 2         # 66: v cols + ones col (softmax sum) + v-rowsum col
ALU = mybir.AluOpType
FT = mybir.ActivationFunctionType
AXX = mybir.AxisListType.X

f32 = mybir.dt.float32
f32r = mybir.dt.float32r
bf16 = mybir.dt.bfloat16

_CACHE = {}


def _build():
    nc = bacc.Bacc(
        "TRN2", target_bir_lowering=False, debug=False, enable_asserts=False,
        num_devices=8,
    )
    hs_d = nc.dram_tensor("hidden_states", [S, D], f32, kind="ExternalInput").ap()
    enc_d = nc.dram_tensor("encoder_hidden_states", [T, C], f32,
                           kind="ExternalInput").ap()
    ip_d = nc.dram_tensor("ip_hidden_states", [P_IP, C], f32,
                          kind="ExternalInput").ap()
    wq_d = nc.dram_tensor("w_q", [D, D], f32r, kind="ExternalInput").ap()
    wk_d = nc.dram_tensor("w_k", [C, D], f32r, kind="ExternalInput").ap()
    wv_d = nc.dram_tensor("w_v", [C, D], f32r, kind="ExternalInput").ap()
    wkip_d = nc.dram_tensor("w_k_ip", [C, D], f32r, kind="ExternalInput").ap()
    wvip_d = nc.dram_tensor("w_v_ip", [C, D], f32r, kind="ExternalInput").ap()
    wout_d = nc.dram_tensor("w_out", [D, D], f32, kind="ExternalInput").ap()
    bout_d = nc.dram_tensor("b_out", [D], f32, kind="ExternalInput").ap()
    out_d = nc.dram_tensor("out", [S, D], f32, kind="ExternalOutput").ap()

    with tile.TileContext(nc) as tc, ExitStack() as ctx:
        n = tc.nc
        const = ctx.enter_context(tc.tile_pool(name="const", bufs=1))
        wq_sb = const.tile([128, KD, D], f32r)
        wout_sb = const.tile([128, KD, D], bf16)
        ktc_sb = const.tile([128, KD, CAT], bf16)
        vaug_sb = const.tile([128, H, VW], bf16)   # rows 0:77 hold txt v
        ipv_sb = const.tile([128, H, VW], bf16)    # rows 96:112 hold ip v
        ident = const.tile([128, 128], f32)
        identb = const.tile([128, 128], bf16)
        onesb_col = const.tile([128, 1], bf16)
        b_row = const.tile([1, D], f32)
        gw128 = const.tile([128, D], f32r)  # row0=-colsum(w_out), row1=b_out

        make_identity(n, ident[:])
        make_identity(n, identb[:])
        n.vector.memset(onesb_col[:], 1.0)
        n.vector.memset(ktc_sb[:, :, T:IPOFF], 0.0)
        n.vector.memset(vaug_sb[0:T, :, HD:HD + 1], 1.0)
        n.vector.memset(ipv_sb[IPOFF:IPOFF + P_IP, :, HD:HD + 1], 1.0)
        n.vector.memset(gw128.bitcast(f32)[:], 0.0)
        n.sync.dma_start(wq_sb[:], wq_d.rearrange("(ko ki) m -> ki ko m", ki=128))
        n.sync.dma_start(b_row[:], bout_d[None, :])
        n.vector.tensor_copy(gw128.bitcast(f32)[1:2, :], b_row[:])

        # ---------------- setup: weights, k/v projections -------------------
        with tc.tile_pool(name="setup", bufs=1) as setup, \
             tc.tile_pool(name="setup_w", bufs=3) as setup_w:
          with tc.tile_pool(name="sps1", bufs=2, space="PSUM") as sps1:
            # wout: load f32 (two halves), cast to bf16
            for half in range(2):
                wout_f = setup.tile([128, KD // 2, D], f32, tag="woutf")
                n.sync.dma_start(
                    wout_f[:],
                    wout_d.rearrange("(ko ki) m -> ki ko m", ki=128)[
                        :, ds(half * (KD // 2), KD // 2), :])
                for k in range(KD // 2):
                    n.scalar.copy(wout_sb[:, half * (KD // 2) + k, :],
                                  wout_f[:, k, :])
            # column sums of w_out for the -g*wsum fold
            for j in range(3):
                w = min(512, D - j * 512)
                wsp = sps1.tile([1, 512], f32, tag="wsp", bufs=1)
                for k in range(KD):
                    n.tensor.matmul(wsp[:, :w], onesb_col[:],
                                    wout_sb[:, k, ds(j * 512, w)],
                                    start=(k == 0), stop=(k == KD - 1))
                n.vector.tensor_scalar_mul(
                    gw128.bitcast(f32)[0:1, ds(j * 512, w)], wsp[:, :w], -1.0)

            enc_sb = setup.tile([T, C], f32, tag="enc")
            n.sync.dma_start(enc_sb[:], enc_d)
            encT = setup.tile([128, KC, T], f32r, tag="encT")
            for c in range(KC):
                tp = sps1.tile([128, T], f32, tag="tp")
                n.tensor.transpose(tp[:], enc_sb[:, ts(c, 128)], ident[:T, :T])
                n.vector.tensor_copy(encT[:, c, :], tp[:])
            ipx_sb = setup.tile([P_IP, C], f32, tag="ipx")
            n.sync.dma_start(ipx_sb[:], ip_d)
            ipT = setup.tile([128, KC, P_IP], f32r, tag="ipT")
            for c in range(KC):
                tp = sps1.tile([128, T], f32, tag="tp")
                n.tensor.transpose(tp[:, :P_IP], ipx_sb[:, ts(c, 128)],
                                   ident[:P_IP, :P_IP])
                n.vector.tensor_copy(ipT[:, c, :], tp[:, :P_IP])

            # ktc (bf16): txt k at cols 0:77, ip k at cols 96:112
            for dt_ in range(KD):
                wk_t = setup_w.tile([128, KC, 128], f32r, tag="wk")
                n.sync.dma_start(
                    wk_t[:],
                    wk_d.rearrange("(co ci) m -> ci co m", ci=128)[:, :, ts(dt_, 128)])
                kp = sps1.tile([128, T], f32, tag="kp")
                for c in range(KC):
                    n.tensor.matmul(kp[:], wk_t[:, c, :].bitcast(f32),
                                    encT[:, c, :].bitcast(f32),
                                    start=(c == 0), stop=(c == KC - 1))
                n.vector.tensor_copy(ktc_sb[:, dt_, 0:T], kp[:])
                wkip_t = setup_w.tile([128, KC, 128], f32r, tag="wk")
                n.sync.dma_start(
                    wkip_t[:],
                    wkip_d.rearrange("(co ci) m -> ci co m", ci=128)[:, :, ts(dt_, 128)])
                kp2 = sps1.tile([128, T], f32, tag="kp")
                for c in range(KC):
                    n.tensor.matmul(kp2[:, :P_IP], wkip_t[:, c, :].bitcast(f32),
                                    ipT[:, c, :].bitcast(f32),
                                    start=(c == 0), stop=(c == KC - 1))
                n.vector.tensor_copy(ktc_sb[:, dt_, IPOFF:IPOFF + P_IP],
                                     kp2[:, :P_IP])

          with tc.tile_pool(name="sps2", bufs=1, space="PSUM") as sps2:
            if True:
                vp = sps2.tile([T, 3 * 512], f32, tag="vp")
                ivp = sps2.tile([128, 3 * 512], f32, tag="ivp")
                for c in range(KC):
                    wv_c = setup_w.tile([128, D], f32r, tag="wv")
                    n.sync.dma_start(
                        wv_c[:],
                        wv_d.rearrange("(co ci) m -> ci co m", ci=128)[:, c, :])
                    wvip_c = setup_w.tile([128, D], f32r, tag="wv")
                    n.sync.dma_start(
                        wvip_c[:],
                        wvip_d.rearrange("(co ci) m -> ci co m", ci=128)[:, c, :])
                    for j in range(3):
                        w = min(512, D - j * 512)
                        n.tensor.matmul(vp[:, ds(j * 512, w)], encT[:, c, :],
                                        wv_c[:, ds(j * 512, w)],
                                        start=(c == 0), stop=(c == KC - 1))
                        n.tensor.matmul(
                            ivp[IPOFF:IPOFF + P_IP, ds(j * 512, w)], ipT[:, c, :],
                            wvip_c[:, ds(j * 512, w)],
                            start=(c == 0), stop=(c == KC - 1),
                            tile_position=(0, IPOFF))
                # scatter [77, 1280] -> vaug [77, 20, 0:64]
                n.vector.tensor_copy(
                    vaug_sb[0:T, :, 0:HD],
                    vp[:, :D].rearrange("p (h c) -> p h c", c=HD))
                n.vector.tensor_copy(
                    ipv_sb[IPOFF:IPOFF + P_IP, :, 0:HD],
                    ivp[IPOFF:IPOFF + P_IP, :D].rearrange("p (h c) -> p h c", c=HD))
                with n.allow_low_precision(reason="v row-sums feed small mean "
                                           "correction; bf16 is plenty"):
                    n.vector.reduce_sum(
                        vaug_sb[0:T, :, HD + 1:HD + 2],
                        vp[:, :D].rearrange("p (h c) -> p h c", c=HD),
                        axis=AXX)
                    n.vector.reduce_sum(
                        ipv_sb[IPOFF:IPOFF + P_IP, :, HD + 1:HD + 2],
                        ivp[IPOFF:IPOFF + P_IP, :D].rearrange(
                            "p (h c) -> p h c", c=HD),
                        axis=AXX)

        # ---------------- main loop over s-blocks --------------------------
        lp = ctx.enter_context(tc.tile_pool(name="lp", bufs=2))
        lp1 = ctx.enter_context(tc.tile_pool(name="lp1", bufs=2))
        lps = ctx.enter_context(tc.tile_pool(name="lps", bufs=2))
        lpp = ctx.enter_context(tc.tile_pool(name="lpp", bufs=12))
        lpo = ctx.enter_context(tc.tile_pool(name="lpo", bufs=2))
        # three disjoint psum rings so consecutive blocks pipeline:
        #  trmm: hs transposes + q-proj (early stages)
        #  scpv: scores + PV (mid stages)
        #  late: gcol transpose + combined transposes + out-proj (late stages)
        ps_a = ctx.enter_context(tc.tile_pool(name="ps_a", bufs=3, space="PSUM"))
        ps_b = ctx.enter_context(tc.tile_pool(name="ps_b", bufs=3, space="PSUM"))
        ps_c = ctx.enter_context(tc.tile_pool(name="ps_c", bufs=2, space="PSUM"))

        PVH = (7, 7, 6)  # heads per 1-bank pv tile
        PVO = (0, 7, 14)
        i32 = mybir.dt.int32

        for b in range(NBLK):
            s0 = b * SB
            # ---- load + transpose hs -> hsT [d, s] (f32) ----
            hsT = lp1.tile([128, KD, SB], f32r, tag="hsT")
            hs_t = {}
            for si in range(2):
                hs_t[si] = lp.tile([128, D], f32, tag=f"hs{si}", name=f"hs{si}")
                n.sync.dma_start(hs_t[si][:], hs_d[ds(s0 + si * 128, 128), :])
            for gi, dp in enumerate(range(0, KD, 2)):
                tp = ps_a.tile([128, 512], f32, tag="trmm")
                for dd in range(2):
                    for si in range(2):
                        n.tensor.transpose(tp.bitcast(f32r)[
                                               :, ds(dd * 256 + si * 128, 128)],
                                           hs_t[si].bitcast(f32r)[
                                               :, ts(dp + dd, 128)],
                                           ident.bitcast(f32r)[:])
                tpv = tp[:].rearrange("p (a b) -> p a b", a=2)
                if gi % 2 == 0:
                    n.scalar.copy(hsT.bitcast(f32)[:, dp:dp + 2, :], tpv)
                else:
                    n.vector.tensor_copy(hsT.bitcast(f32)[:, dp:dp + 2, :], tpv)

            # ---- qT [d, s] (bf16) ----
            qT = lp1.tile([128, KD, SB], bf16, tag="qT")
            for dp in range(0, KD, 2):
                qp = ps_a.tile([128, 512], f32, tag="trmm")
                for dd in range(2):
                    for k in range(KD):
                        n.tensor.matmul(qp[:, ds(dd * SB, SB)],
                                        wq_sb[:, k, ts(dp + dd, 128)], hsT[:, k, :],
                                        start=(k == 0), stop=(k == KD - 1))
                n.scalar.copy(qT[:, dp:dp + 2, :],
                              qp[:].rearrange("p (a b) -> p a b", a=2))

            # ---- scores + exp per head-pair: [112, 512] ----
            pTc = {}
            for hp in range(KD):
                sc = ps_b.tile([128, 512], f32, tag="scpv", name=f"sc{hp}")
                for half in range(2):
                    n.tensor.matmul(sc[0:CAT, ds(half * SB, SB)],
                                    ktc_sb[ds(64 * half, 64), hp, :],
                                    qT[ds(64 * half, 64), hp, :],
                                    start=True, stop=True)
                pTc[hp] = lpp.tile([CAT, 512], bf16, tag="pTc", name=f"pTc{hp}")
                n.scalar.activation(pTc[hp][:], sc[0:CAT, :], FT.Exp, scale=SCALE)

            # ---- PV + normalize ----
            lat_n = lp1.tile([128, 2, H, HD], bf16, tag="lat_n")
            ipo_n = lp1.tile([128, 2, H, HD], bf16, tag="ipo_n")
            sm = lps.tile([128, 2, 2, H, 2], f32, tag="sm")
            recip = lps.tile([128, 2, 2, H], f32, tag="recip")
            msum = lps.tile([128, 2, 2, H], f32, tag="msum")
            st = lps.tile([128, 2, 24], f32, tag="st")
            sq_scr = lps.tile([128, H, HD], bf16, tag="sqscr")

            for si in range(2):
                for br in range(2):  # 0 = txt, 1 = ip
                    dest = lat_n if br == 0 else ipo_n
                    for g in range(3):
                        nh, h0 = PVH[g], PVO[g]
                        pv = ps_b.tile([128, 512], f32, tag="scpv",
                                       name=f"pv{g}")
                        for j in range(nh):
                            h = h0 + j
                            hp, half = h // 2, h % 2
                            col0 = half * SB + si * 128
                            if br == 0:
                                lhsT = pTc[hp][0:T, ds(col0, 128)]
                                rhs = vaug_sb[0:T, h, :]
                            else:
                                lhsT = pTc[hp][IPOFF:IPOFF + P_IP, ds(col0, 128)]
                                rhs = ipv_sb[IPOFF:IPOFF + P_IP, h, :]
                            n.tensor.matmul(pv[:, ds(j * VW, VW)],
                                            lhsT, rhs, start=True, stop=True,
                                            tile_position=(
                                                (IPOFF, 0) if br else (0, 0)))
                        pvj = pv[:, 0:nh * VW].rearrange("p (j c) -> p j c", c=VW)
                        n.vector.tensor_copy(sm[:, si, br, ds(h0, nh), :],
                                             pvj[:, :, HD:HD + 2])
                        n.vector.reciprocal(recip[:, si, br, ds(h0, nh)],
                                            sm[:, si, br, ds(h0, nh), 0])
                        n.vector.tensor_mul(msum[:, si, br, ds(h0, nh)],
                                            sm[:, si, br, ds(h0, nh), 1],
                                            recip[:, si, br, ds(h0, nh)])
                        n.vector.tensor_tensor(
                            dest[:, si, ds(h0, nh), :],
                            pvj[:, :, 0:HD],
                            recip[:, si, br, ds(h0, nh), None].to_broadcast(
                                [128, nh, HD]),
                            op=ALU.mult)

            # ---- norm_ipa stats ----
            for si in range(2):
                n.vector.reduce_sum(st[:, si, 0:1], msum[:, si, 0, :], axis=AXX)
                n.vector.reduce_sum(st[:, si, 1:2], msum[:, si, 1, :], axis=AXX)
                n.vector.tensor_tensor_reduce(
                    out=sq_scr[:], in0=lat_n[:, si], in1=lat_n[:, si],
                    scale=1.0, scalar=0.0, op0=ALU.mult, op1=ALU.add,
                    accum_out=st[:, si, 2:3])
                n.vector.tensor_tensor_reduce(
                    out=sq_scr[:], in0=ipo_n[:, si], in1=ipo_n[:, si],
                    scale=1.0, scalar=0.0, op0=ALU.mult, op1=ALU.add,
                    accum_out=st[:, si, 3:4])
                # means and variances
                n.vector.tensor_scalar_mul(st[:, si, 4:5], st[:, si, 0:1], 1.0 / D)
                n.vector.tensor_scalar_mul(st[:, si, 5:6], st[:, si, 1:2], 1.0 / D)
                n.vector.tensor_mul(st[:, si, 6:7], st[:, si, 4:5], st[:, si, 4:5])
                n.vector.tensor_mul(st[:, si, 7:8], st[:, si, 5:6], st[:, si, 5:6])
                n.vector.tensor_scalar(out=st[:, si, 8:9], in0=st[:, si, 2:3],
                                       scalar1=1.0 / D, scalar2=st[:, si, 6:7],
                                       op0=ALU.mult, op1=ALU.subtract)
                n.vector.tensor_scalar(out=st[:, si, 9:10], in0=st[:, si, 3:4],
                                       scalar1=1.0 / D, scalar2=st[:, si, 7:8],
                                       op0=ALU.mult, op1=ALU.subtract)
            # std = sqrt(var) via DVE: fast-inverse-sqrt init + 3 Newton
            # iters (keeps ACT on the Exp table all kernel long)
            vv = st[:, :, 8:10]
            yy = st[:, :, 10:12]
            t0 = st[:, :, 12:14]
            n.vector.tensor_scalar(out=yy.bitcast(i32), in0=vv.bitcast(i32),
                                   scalar1=1, scalar2=None,
                                   op0=ALU.logical_shift_right)
            n.vector.tensor_scalar(out=yy.bitcast(i32), in0=yy.bitcast(i32),
                                   scalar1=-1, scalar2=0x5f3759df,
                                   op0=ALU.mult, op1=ALU.add)
            for _ in range(3):
                n.vector.tensor_mul(t0[:], yy[:], yy[:])
                n.vector.tensor_mul(t0[:], t0[:], vv[:])
                n.vector.tensor_scalar(out=t0[:], in0=t0[:], scalar1=-0.5,
                                       scalar2=1.5, op0=ALU.mult, op1=ALU.add)
                n.vector.tensor_mul(yy[:], yy[:], t0[:])
            # y ~= rsqrt(var); std = var * y
            n.vector.tensor_mul(yy[:], vv[:], yy[:])
            hsT2 = lp1.tile([128, KD, SB], bf16, tag="hsT2")
            gp = lps.tile([128, 2, 128], f32r, tag="gp")
            for si in range(2):
                n.vector.tensor_scalar_add(st[:, si, 12:13], st[:, si, 11:12], EPS)
                n.vector.reciprocal(st[:, si, 13:14], st[:, si, 12:13])
                n.vector.tensor_mul(st[:, si, 14:15], st[:, si, 10:11],
                                    st[:, si, 13:14])
                # gneg = alpha*mean_ip - mean_lat; gcol = [gneg, 1.0]
                n.vector.scalar_tensor_tensor(
                    out=st[:, si, 15:16], in0=st[:, si, 5:6],
                    scalar=st[:, si, 14:15], in1=st[:, si, 4:5],
                    op0=ALU.mult, op1=ALU.subtract)
                n.vector.memset(st[:, si, 16:17], 1.0)
                gt = ps_c.tile([128, 512], f32, tag="late", name="gt")
                n.tensor.transpose(gt[0:2, 0:128], st[:, si, 15:17], ident[:])
                n.vector.tensor_copy(gp.bitcast(f32)[0:2, si, :], gt[0:2, 0:128])
                # combine in place: lat_n <- lat_n + alpha * ipo_n
                n.gpsimd.scalar_tensor_tensor(
                    out=lat_n[:, si].rearrange("p h c -> p (h c)"),
                    in0=ipo_n[:, si].rearrange("p h c -> p (h c)"),
                    scalar=st[:, si, 14:15],
                    in1=lat_n[:, si].rearrange("p h c -> p (h c)"),
                    op0=ALU.mult, op1=ALU.add)

            # ---- transpose combined -> hsT2 [d, s] (bf16) ----
            for gi, dp in enumerate(range(0, KD, 2)):
                tp = ps_c.tile([128, 512], bf16, tag="late", name="tp2")
                for dd in range(2):
                    for si in range(2):
                        n.tensor.transpose(
                            tp[:, ds(dd * 256 + si * 128, 128)],
                            lat_n[:, si].rearrange(
                                "p h c -> p (h c)")[:, ts(dp + dd, 128)],
                            identb[:])
                n.vector.tensor_copy(hsT2[:, dp:dp + 2, :],
                                     tp[:].rearrange("p (a b) -> p a b", a=2))

            # ---- out projection (+ bias - g*wsum via f32r matmul) ----
            for si in range(2):
                for j in range(3):
                    w = min(512, D - j * 512)
                    op = ps_c.tile([128, 512], f32, tag="late", name="op")
                    n.tensor.matmul(op[:, :w], gp[:, si, :],
                                    gw128[:, ds(j * 512, w)],
                                    start=True, stop=False)
                    for k in range(KD):
                        n.tensor.matmul(op[:, :w], hsT2[:, k, ts(si, 128)],
                                        wout_sb[:, k, ds(j * 512, w)],
                                        start=False, stop=(k == KD - 1))
                    ost = lpo.tile([128, 512], f32, tag="ost")
                    n.scalar.copy(ost[:, :w], op[:, :w])
                    n.sync.dma_start(
                        out_d[ds(s0 + si * 128, 128), ds(j * 512, w)], ost[:, :w])
    nc.compile()
    return nc


def _get_nc():
    if "nc" not in _CACHE:
        _CACHE["nc"] = _build()
    return _CACHE["nc"]


def kernel(**inputs) -> np.ndarray:
    nc = _get_nc()
    f = lambda x: np.ascontiguousarray(np.asarray(x), dtype=np.float32)
    shared = {k: f(inputs[k]) for k in
              ("w_q", "w_k", "w_v", "w_k_ip", "w_v_ip", "w_out", "b_out")}
    hs = f(inputs["hidden_states"])
    enc = f(inputs["encoder_hidden_states"])
    ipx = f(inputs["ip_hidden_states"])
    in_maps = [
        dict(shared, hidden_states=hs[i], encoder_hidden_states=enc[i],
             ip_hidden_states=ipx[i])
        for i in range(8)
    ]
    res = bass_utils.run_bass_kernel_spmd(nc, in_maps, core_ids=list(range(8)))
    return np.stack([res.results[i]["out"] for i in range(8)], axis=0)


if __name__ == "__main__":
    rng = np.random.default_rng(0)
    ins = {
        "hidden_states": rng.standard_normal((B, S, D), dtype=np.float32),
        "encoder_hidden_states": rng.standard_normal((B, T, C), dtype=np.float32),
        "ip_hidden_states": rng.standard_normal((B, P_IP, C), dtype=np.float32),
        "w_q": (rng.standard_normal((D, D), dtype=np.float32) * 0.02),
        "w_k": (rng.standard_normal((C, D), dtype=np.float32) * 0.02),
        "w_v": (rng.standard_normal((C, D), dtype=np.float32) * 0.02),
        "w_k_ip": (rng.standard_normal((C, D), dtype=np.float32) * 0.02),
        "w_v_ip": (rng.standard_normal((C, D), dtype=np.float32) * 0.02),
        "w_out": (rng.standard_normal((D, D), dtype=np.float32) * 0.02),
        "b_out": np.zeros((D,), dtype=np.float32),
    }
    out = kernel(**ins)
    print("out", out.shape, out.dtype, float(np.abs(out).max()))
